# revision 9
# baseline (speedup 1.0000x reference)
"""DHT transform kernel for Trainium2 (Bass/Tile), 8-core data parallel.

Problem: given x [B=2e6, 1] fp32, produce out [B, 4, 4] where
  out[b] = T_theta(x_b) @ RIGHT,
  T_theta = [[c,-s,0,0],[s,c,0,0],[0,0,1,0],[0,0,0,1]],  c=cos(x_b), s=sin(x_b)
  RIGHT   = T_d @ T_a @ T_alpha (constant 4x4).

Rows 2,3 of every output matrix are input-independent constants; rows 0,1
are 8 scalar multiples of cos(x)/sin(x):
  row0 = [ c,     -s*ca,  s*sa,  A*c ]
  row1 = [ s,      c*ca, -c*sa,  A*s ]
  row2 = [ 0,      sa,    ca,    D   ]      (constant)
  row3 = [ 0,      0,     0,     1   ]      (constant)

v6 strategy (memory-regime: minimize device HBM bytes, then overlap):
  - device I/O in fp16: in 0.5 MB + out 4.0 MB per core (vs 17 MB fp32
    interleaved).  fp16 rounding keeps absmax err ~3e-3 << the 2e-2 gate.
    (int8 store via SWDGE cast-DMA was tried and is a net loss: the SDMA
    read side still moves fp16 bytes, and SWDGE adds issue + drain cost.)
  - device writes ONLY the 8 variable slots, in slot-major per-tile
    blocks [p, 8, f] (all ACT/DVE writes contiguous, no strided ops, no
    const-slot memsets).  Host unshard de-interleaves blocks, upcasts to
    fp32, and fills the 8 constant slots.
  - half-angle scheme (AF.Sin is only valid on |arg|<=pi; |x|<5.4):
    g = Sin(x/4), h = Sin(x/2) on ACT; s=h^2, u=g^2, v=2-4u on DVE;
    ct-family blocks as affine tensor_scalar directly from s (ct=1-2s
    folded per coefficient); st block = h*v; st-products split DVE/ACT.
  - x loaded up front in 3 chunks (own completion sems) so compute never
    waits on a bulk load; 4 tiles (DVE per-tile overhead) with small
    head/tail to hide ramp and drain; 4 out buffers so no WAR stalls.
"""

import numpy as np

import concourse.bass as bass
import concourse.bacc as bacc
import concourse.tile as tile
import concourse.mybir as mybir
from concourse.bass_utils import run_bass_kernel_spmd

F32 = mybir.dt.float32
F16 = mybir.dt.float16
AF = mybir.ActivationFunctionType
ALU = mybir.AluOpType

# ---------------- problem constants (hardcoded) ----------------
B_TOTAL = 2_000_000
N_CORES = 8
PER_CORE = B_TOTAL // N_CORES          # 250_000
P = 128                                # SBUF partitions
F_TILES = (128, 704, 768, 360)         # taper: small head (ramp) + tail (drain)
W = sum(F_TILES)                       # 1960; 128*1960 = 250880 >= 250000
PADDED = P * W


def _right_chain() -> np.ndarray:
    # replicate reference's fp32 constant chain exactly
    d_val, a_val, alpha = np.float32(0.1), np.float32(0.2), np.float32(0.3)
    d_mat = np.array([[0,0,0,0],[0,0,0,0],[0,0,0,1],[0,0,0,0]], np.float32)
    a_mat = np.array([[0,0,0,1],[0,0,0,0],[0,0,0,0],[0,0,0,0]], np.float32)
    al_cos = np.array([[0,0,0,0],[0,1,0,0],[0,0,1,0],[0,0,0,0]], np.float32)
    al_sin = np.array([[0,0,0,0],[0,0,-1,0],[0,1,0,0],[0,0,0,0]], np.float32)
    al_const = np.array([[1,0,0,0],[0,0,0,0],[0,0,0,0],[0,0,0,1]], np.float32)
    t_d = d_mat * d_val + np.eye(4, dtype=np.float32)
    t_a = a_mat * a_val + np.eye(4, dtype=np.float32)
    t_alpha = al_cos * np.cos(alpha) + al_sin * np.sin(alpha) + al_const
    return t_d @ t_a @ t_alpha


_R = _right_chain()
_CA = float(_R[1, 1])   # cos(alpha)
_SA = float(_R[2, 1])   # sin(alpha)
_AV = float(_R[0, 3])   # a
_DV = float(_R[2, 3])   # d

# ct-family slots: blk j = c*ct = c - 2c*s  (affine in s = h^2)
_CT_SLOTS = ((0, 1.0), (3, _AV), (5, _CA), (6, -_SA))
# st-family slots from blk4 (= st): DVE takes 2, ACT takes 1 (balance)
_ST_DVE = ((1, -_CA), (2, _SA))
_ST_ACT = ((7, _AV),)
# constant slots 8..15 filled host-side:
_CONST_TAIL = np.array([0.0, _SA, _CA, _DV, 0.0, 0.0, 0.0, 1.0], np.float32)


def _build_nc():
    nc = bacc.Bacc(
        None, target_bir_lowering=False, debug=False, num_devices=N_CORES
    )
    x_ext = nc.declare_dram_parameter("x", [P, W], F16, isOutput=False)
    out_ext = nc.declare_dram_parameter("out", [P, W * 8], F16, isOutput=True)
    fmax = max(F_TILES)
    f0, f1 = F_TILES[0], F_TILES[1]

    with tile.TileContext(nc) as tc:
        with (
            tc.tile_pool(name="xin", bufs=1) as xin_pool,
            tc.tile_pool(name="io", bufs=4) as io_pool,
            tc.tile_pool(name="tmp", bufs=3) as tmp_pool,
        ):
            # x resident in SBUF as three tiles with separate completion
            # sems: tiny head (tile-0 compute starts ~2 us post-barrier),
            # tile-1 chunk, bulk remainder.
            xh = xin_pool.tile([P, f0], F16, tag="xh")
            nc.sync.dma_start(xh[:], x_ext[:, :f0])
            x1 = xin_pool.tile([P, f1], F16, tag="x1")
            nc.sync.dma_start(x1[:], x_ext[:, f0 : f0 + f1])
            xr = xin_pool.tile([P, W - f0 - f1], F16, tag="xr")
            nc.sync.dma_start(xr[:], x_ext[:, f0 + f1 :])

            off = 0
            for t, f in enumerate(F_TILES):
                if t == 0:
                    xs = xh[:]
                elif t == 1:
                    xs = x1[:]
                else:
                    xs = xr[:, off - f0 - f1 : off - f0 - f1 + f]
                ob = io_pool.tile([P, fmax * 8], F16, tag="ob")

                def blk(j):
                    return ob[:, j * f : (j + 1) * f]

                # ACT: the two sines
                g = tmp_pool.tile([P, fmax], F16, tag="g")
                nc.scalar.activation(g[:, :f], xs, AF.Sin, scale=0.25)
                h = tmp_pool.tile([P, fmax], F16, tag="h")
                nc.scalar.activation(h[:, :f], xs, AF.Sin, scale=0.5)

                # DVE (fp16 perf modes): u=g^2, s=h^2 (TT); v=2-4u (TS);
                # ct-family straight from s (TS affine); st=h*v (TT);
                # 2 st-products (TS); 1 st-product on ACT
                u = tmp_pool.tile([P, fmax], F16, tag="u")
                nc.vector.tensor_mul(u[:, :f], g[:, :f], g[:, :f])
                s = tmp_pool.tile([P, fmax], F16, tag="s")
                nc.vector.tensor_mul(s[:, :f], h[:, :f], h[:, :f])
                v = tmp_pool.tile([P, fmax], F16, tag="v")
                nc.vector.tensor_scalar(
                    v[:, :f], u[:, :f], -4.0, 2.0, ALU.mult, ALU.add
                )
                for j, c in _CT_SLOTS:
                    nc.vector.tensor_scalar(
                        blk(j), s[:, :f], float(-2.0 * c), float(c),
                        ALU.mult, ALU.add,
                    )
                nc.vector.tensor_mul(blk(4), h[:, :f], v[:, :f])
                for j, c in _ST_DVE:
                    nc.vector.tensor_scalar_mul(blk(j), blk(4), float(c))
                for j, c in _ST_ACT:
                    nc.scalar.mul(blk(j), blk(4), float(c))

                nc.sync.dma_start(
                    out_ext[:, off * 8 : (off + f) * 8], ob[:, : f * 8]
                )
                off += f
    nc.compile()
    return nc


_NC_CACHE = {}


def _get_nc():
    if "nc" not in _NC_CACHE:
        _NC_CACHE["nc"] = _build_nc()
    return _NC_CACHE["nc"]


def _make_in_maps(x: np.ndarray) -> list:
    flat = np.ascontiguousarray(x.reshape(-1)).astype(np.float16)
    # padded overlapping shards: core k handles [k*PER_CORE, k*PER_CORE+PADDED)
    in_maps = []
    for k in range(N_CORES):
        start = k * PER_CORE
        end = start + PADDED
        if end <= B_TOTAL:
            shard = flat[start:end]
        else:
            shard = np.concatenate(
                [flat[start:], np.zeros(end - B_TOTAL, np.float16)]
            )
        in_maps.append({"x": shard.reshape(P, W)})
    return in_maps


def kernel(x: np.ndarray) -> np.ndarray:
    assert x.shape == (B_TOTAL, 1) and x.dtype == np.float32
    in_maps = _make_in_maps(x)
    nc = _get_nc()
    res = run_bass_kernel_spmd(nc, in_maps, list(range(N_CORES)))

    out = np.empty((B_TOTAL, 16), np.float32)
    arr = np.empty((P, W, 8), np.float16)
    for k in range(N_CORES):
        part = res.results[k]["out"]  # [P, W*8] fp16, slot-major per tile
        off = 0
        for f in F_TILES:
            blk = part[:, off * 8 : (off + f) * 8].reshape(P, 8, f)
            arr[:, off : off + f, :] = blk.transpose(0, 2, 1)
            off += f
        out[k * PER_CORE : (k + 1) * PER_CORE, :8] = arr.reshape(PADDED, 8)[
            :PER_CORE
        ]
    out[:, 8:] = _CONST_TAIL
    return out.reshape(B_TOTAL, 4, 4)


# revision 10
# speedup vs baseline: 1.1815x; 1.1815x over previous
"""DHT transform kernel for Trainium2 (Bass/Tile), 8-core data parallel.

Problem: given x [B=2e6, 1] fp32, produce out [B, 4, 4] where
  out[b] = T_theta(x_b) @ RIGHT,
  T_theta = [[c,-s,0,0],[s,c,0,0],[0,0,1,0],[0,0,0,1]],  c=cos(x_b), s=sin(x_b)
  RIGHT   = T_d @ T_a @ T_alpha (constant 4x4).

Rows 2,3 of every output matrix are input-independent constants; rows 0,1
are 8 scalar multiples of cos(x)/sin(x):
  row0 = [ c,     -s*ca,  s*sa,  A*c ]
  row1 = [ s,      c*ca, -c*sa,  A*s ]
  row2 = [ 0,      sa,    ca,    D   ]      (constant)
  row3 = [ 0,      0,     0,     1   ]      (constant)

v7 strategy (memory-regime: minimize device HBM bytes, then overlap):
  - device I/O in fp16: in 0.5 MB + out 4.0 MB per core.  fp16 rounding
    keeps absmax err ~3e-3 << the 2e-2 gate.  (int8 store via SWDGE
    cast-DMA was tried and lost: the SDMA read side still moves fp16.)
  - device writes ONLY the 8 variable slots as contiguous per-tile
    blocks; host unshard de-interleaves, upcasts, fills constant slots.
  - half-angle scheme (AF.Sin valid only on |arg|<=pi; |x|<5.4):
    ACT is a pure Sin factory (g=Sin(x/4), h=Sin(x/2) into one [g|h]
    strip); EVERYTHING else on DVE fp16 perf modes: [u|s]=[g|h]^2 in one
    2f-wide TT, ct-family as affine TS from s, v=2-4u, st=h*v, st
    products as TS.  No cross-engine product deps: ACT never waits on
    DVE, so the Sin stream feeds tiles without coupling stalls (v4/v6
    showed 2-4 us ACT waits when a product lived on ACT).
  - per-tile output DMA split in two: ct-half [c,Ac,ca*c,-sa*c] issues
    mid-tile while DVE computes the st-half [s,-ca*s,sa*s,As] (block
    order in SBUF/DRAM is (0,3,5,6,4,1,2,7); host permutes back).
  - x loaded up front in 3 chunks (own completion sems); small head and
    tail tiles hide pipeline ramp and drain; 4 rotating out buffers.
"""

import numpy as np

import concourse.bass as bass
import concourse.bacc as bacc
import concourse.tile as tile
import concourse.mybir as mybir
from concourse.bass_utils import run_bass_kernel_spmd

F32 = mybir.dt.float32
F16 = mybir.dt.float16
AF = mybir.ActivationFunctionType
ALU = mybir.AluOpType

# ---------------- problem constants (hardcoded) ----------------
B_TOTAL = 2_000_000
N_CORES = 8
PER_CORE = B_TOTAL // N_CORES          # 250_000
P = 128                                # SBUF partitions
F_TILES = (128, 544, 544, 544, 200)    # taper: small head (ramp) + tail (drain)
W = sum(F_TILES)                       # 1960; 128*1960 = 250880 >= 250000
PADDED = P * W


def _right_chain() -> np.ndarray:
    # replicate reference's fp32 constant chain exactly
    d_val, a_val, alpha = np.float32(0.1), np.float32(0.2), np.float32(0.3)
    d_mat = np.array([[0,0,0,0],[0,0,0,0],[0,0,0,1],[0,0,0,0]], np.float32)
    a_mat = np.array([[0,0,0,1],[0,0,0,0],[0,0,0,0],[0,0,0,0]], np.float32)
    al_cos = np.array([[0,0,0,0],[0,1,0,0],[0,0,1,0],[0,0,0,0]], np.float32)
    al_sin = np.array([[0,0,0,0],[0,0,-1,0],[0,1,0,0],[0,0,0,0]], np.float32)
    al_const = np.array([[1,0,0,0],[0,0,0,0],[0,0,0,0],[0,0,0,1]], np.float32)
    t_d = d_mat * d_val + np.eye(4, dtype=np.float32)
    t_a = a_mat * a_val + np.eye(4, dtype=np.float32)
    t_alpha = al_cos * np.cos(alpha) + al_sin * np.sin(alpha) + al_const
    return t_d @ t_a @ t_alpha


_R = _right_chain()
_CA = float(_R[1, 1])   # cos(alpha)
_SA = float(_R[2, 1])   # sin(alpha)
_AV = float(_R[0, 3])   # a
_DV = float(_R[2, 3])   # d

# SBUF/DRAM block position -> output slot (host permutes back):
# ct-half positions 0..3 = slots (0, 3, 5, 6); st-half 4..7 = (4, 1, 2, 7)
_SLOT_ORDER = (0, 3, 5, 6, 4, 1, 2, 7)
# ct-family (position, coeff): blk = c*ct = c - 2c*s  (affine in s = h^2)
_CT_POS = ((0, 1.0), (1, _AV), (2, _CA), (3, -_SA))
# st-family products from position 4 (= st): (position, coeff)
_ST_POS = ((5, -_CA), (6, _SA), (7, _AV))
# constant slots 8..15 filled host-side:
_CONST_TAIL = np.array([0.0, _SA, _CA, _DV, 0.0, 0.0, 0.0, 1.0], np.float32)


def _build_nc():
    nc = bacc.Bacc(
        None, target_bir_lowering=False, debug=False, num_devices=N_CORES
    )
    x_ext = nc.declare_dram_parameter("x", [P, W], F16, isOutput=False)
    out_ext = nc.declare_dram_parameter("out", [P, W * 8], F16, isOutput=True)
    fmax = max(F_TILES)
    f0, f1 = F_TILES[0], F_TILES[1]

    with tile.TileContext(nc) as tc:
        with (
            tc.tile_pool(name="xin", bufs=1) as xin_pool,
            tc.tile_pool(name="io", bufs=4) as io_pool,
            tc.tile_pool(name="tmp", bufs=3) as tmp_pool,
        ):
            # x resident in SBUF as three tiles with separate completion
            # sems: tiny head (tile-0 compute starts ~2 us post-barrier),
            # tile-1 chunk, bulk remainder.
            xh = xin_pool.tile([P, f0], F16, tag="xh")
            nc.sync.dma_start(xh[:], x_ext[:, :f0])
            x1 = xin_pool.tile([P, f1], F16, tag="x1")
            nc.sync.dma_start(x1[:], x_ext[:, f0 : f0 + f1])
            xr = xin_pool.tile([P, W - f0 - f1], F16, tag="xr")
            nc.sync.dma_start(xr[:], x_ext[:, f0 + f1 :])

            off = 0
            for t, f in enumerate(F_TILES):
                if t == 0:
                    xs = xh[:]
                elif t == 1:
                    xs = x1[:]
                else:
                    xs = xr[:, off - f0 - f1 : off - f0 - f1 + f]
                ob = io_pool.tile([P, fmax * 8], F16, tag="ob")

                def blk(j):
                    return ob[:, j * f : (j + 1) * f]

                # ACT: the two sines into one [g|h] strip
                gh = tmp_pool.tile([P, 2 * fmax], F16, tag="gh")
                nc.scalar.activation(gh[:, :f], xs, AF.Sin, scale=0.25)
                nc.scalar.activation(gh[:, f : 2 * f], xs, AF.Sin, scale=0.5)

                # DVE: [u|s] = [g|h]^2 in one 2f TT; ct-half from s; then
                # v=2-4u, st=h*v, st products; DMA after each half
                us = tmp_pool.tile([P, 2 * fmax], F16, tag="us")
                nc.vector.tensor_mul(us[:, : 2 * f], gh[:, : 2 * f], gh[:, : 2 * f])
                s_ap = us[:, f : 2 * f]
                for j, c in _CT_POS:
                    nc.vector.tensor_scalar(
                        blk(j), s_ap, float(-2.0 * c), float(c),
                        ALU.mult, ALU.add,
                    )
                nc.sync.dma_start(
                    out_ext[:, off * 8 : off * 8 + 4 * f], ob[:, : 4 * f]
                )

                v = tmp_pool.tile([P, fmax], F16, tag="v")
                nc.vector.tensor_scalar(
                    v[:, :f], us[:, :f], -4.0, 2.0, ALU.mult, ALU.add
                )
                nc.vector.tensor_mul(blk(4), gh[:, f : 2 * f], v[:, :f])
                for j, c in _ST_POS:
                    nc.vector.tensor_scalar_mul(blk(j), blk(4), float(c))
                nc.sync.dma_start(
                    out_ext[:, off * 8 + 4 * f : (off + f) * 8],
                    ob[:, 4 * f : 8 * f],
                )
                off += f
    nc.compile()
    return nc


_NC_CACHE = {}


def _get_nc():
    if "nc" not in _NC_CACHE:
        _NC_CACHE["nc"] = _build_nc()
    return _NC_CACHE["nc"]


def _make_in_maps(x: np.ndarray) -> list:
    flat = np.ascontiguousarray(x.reshape(-1)).astype(np.float16)
    # padded overlapping shards: core k handles [k*PER_CORE, k*PER_CORE+PADDED)
    in_maps = []
    for k in range(N_CORES):
        start = k * PER_CORE
        end = start + PADDED
        if end <= B_TOTAL:
            shard = flat[start:end]
        else:
            shard = np.concatenate(
                [flat[start:], np.zeros(end - B_TOTAL, np.float16)]
            )
        in_maps.append({"x": shard.reshape(P, W)})
    return in_maps


def kernel(x: np.ndarray) -> np.ndarray:
    assert x.shape == (B_TOTAL, 1) and x.dtype == np.float32
    in_maps = _make_in_maps(x)
    nc = _get_nc()
    res = run_bass_kernel_spmd(nc, in_maps, list(range(N_CORES)))

    order = list(_SLOT_ORDER)
    out = np.empty((B_TOTAL, 16), np.float32)
    arr = np.empty((P, W, 8), np.float16)
    for k in range(N_CORES):
        part = res.results[k]["out"]  # [P, W*8] fp16, position-major per tile
        off = 0
        for f in F_TILES:
            blk = part[:, off * 8 : (off + f) * 8].reshape(P, 8, f)
            arr[:, off : off + f, order] = blk.transpose(0, 2, 1)
            off += f
        out[k * PER_CORE : (k + 1) * PER_CORE, :8] = arr.reshape(PADDED, 8)[
            :PER_CORE
        ]
    out[:, 8:] = _CONST_TAIL
    return out.reshape(B_TOTAL, 4, 4)
